# revision 3
# baseline (speedup 1.0000x reference)
"""AvgPool2d(64x64, stride 1) with replicate-padding back to (512, 512),
as a distributed Bass kernel on 8 TRN2 NeuronCores.

Input : x (8, 64, 512, 512) float32
Output: (8, 64, 512, 512) float32

Strategy (pure data parallel): one batch element per core. Per core the
pooling is a separable 64-wide box filter; both directions are computed
on the TensorEngine as matmuls against a banded 0/1-matrix `band` of
shape [512, 512] with band[h, i] = 1/64 iff clamp(i-31, 0, 448) <= h <
clamp(i-31, 0, 448) + 64 (the clamp folds the replicate-padding into the
matrix, and the 1/64 folds the averaging).

    V^T = (X^T @ band)        pass 1: vertical box mean, transposed
    O   = (V^T)^T @ band      pass 2: horizontal box mean, natural layout

Both passes put the *data* tile in the stationary (lhsT) operand and the
band in the moving operand, which avoids every transpose.

V3 (memory roofline): the kernel is HBM-bound, so x and out live in DRAM
as bf16 (host casts f32->bf16 on upload, upcasts on download; the kernel
always computed in bf16, the extra output rounding is ~4e-3 rel, within
the 2e-2 gate). That halves HBM traffic to 64 MB/core.

SDMA engines pay ~20 ns fixed per descriptor, so with 1KB (one bf16 row)
descriptors they run at ~18 GB/s instead of ~28 GB/s and the 16-engine
pool becomes the bottleneck. Since the host already permutes/casts, the
DRAM image of both x and out is stored PRE-BLOCKED as
[C/G, 128, G, 4, 512]: partition p's slice of a G=4-channel group is one
contiguous 16KB chunk (descriptor overhead ~3%), while SBUF receives the
standard 128-row block layout (h = 128k + p) that gives the minimal
701-column matmul plans. Input loads ride the sync/SP HWDGE ring, output
stores the scalar/ACT ring; PSUM->SBUF copies alternate Vector/Scalar.
"""

import numpy as np
import ml_dtypes

C, H, W = 64, 512, 512
P = 128
NKH = H // P  # 4 partition blocks
KERNEL = 64
OUT_VALID = H - KERNEL + 1  # 449
PT = (H - OUT_VALID) // 2  # 31 (left/top pad)
G = 4  # channels per DMA group (16KB descriptors)
NG = C // G

# Matmul plan for one PSUM tile, contraction over standard 128-row
# blocks k; each instruction's column range is uniformly "first writer"
# or "accumulating" so per-element PSUM has_written semantics hold:
# (k, lo, hi, start, stop).
MM_PLAN_BLOCK = [
    (0, 0, 159, True, False),
    (1, 96, 159, False, False),
    (1, 159, 287, False, False),
    (2, 224, 287, False, False),
    (2, 287, 415, False, False),
    (3, 352, 415, False, False),
    (3, 415, 512, False, True),
]


def make_band() -> np.ndarray:
    i = np.arange(H)
    ic = np.clip(i - PT, 0, OUT_VALID - 1)
    h = np.arange(H)
    band = (h[:, None] >= ic[None, :]) & (h[:, None] < ic[None, :] + KERNEL)
    return (band.astype(np.float32) / KERNEL).astype(ml_dtypes.bfloat16)


def build_avgpool(tc, x_ap, band_ap, out_ap):
    import concourse.mybir as mybir

    nc = tc.nc
    f32 = mybir.dt.float32
    bf16 = mybir.dt.bfloat16

    with (
        tc.tile_pool(name="const", bufs=1) as const_pool,
        tc.tile_pool(name="xin", bufs=2) as xin_pool,
        tc.tile_pool(name="vt", bufs=4) as vt_pool,
        tc.tile_pool(name="oout", bufs=2) as out_pool,
        tc.tile_pool(name="vtps", bufs=4, space="PSUM") as vt_psum,
        tc.tile_pool(name="ops", bufs=4, space="PSUM") as o_psum,
    ):
        # band in standard block layout: [p, k, i] = band[128*k + p, i]
        band_t = const_pool.tile([P, NKH, H], bf16, tag="band")
        nc.sync.dma_start(band_t[:], band_ap.rearrange("(kh p) i -> p kh i", p=P))

        for g in range(NG):
            # one 16KB-per-partition load for G channels; DRAM already
            # holds the block layout, so this is a straight copy
            xg = xin_pool.tile([P, G, NKH, W], bf16, tag="xg")
            nc.sync.dma_start(xg[:], x_ap[g])

            o_sb = out_pool.tile([P, G, 4, W], bf16, tag="osb")
            for ci in range(G):
                # pass 1: V^T[w, i] = sum_h X[h, w] * band[h, i]
                vtb = vt_pool.tile([P, NKH, H], bf16, tag="vtb")
                for mw in range(NKH):
                    vt_ps = vt_psum.tile([P, H], f32, tag="vt")
                    for k, lo, hi, start, stop in MM_PLAN_BLOCK:
                        nc.tensor.matmul(
                            vt_ps[:, lo:hi],
                            xg[:, ci, k, P * mw : P * (mw + 1)],
                            band_t[:, k, lo:hi],
                            start=start,
                            stop=stop,
                        )
                    if mw % 2 == 0:
                        nc.scalar.copy(vtb[:, mw, :], vt_ps[:])
                    else:
                        nc.vector.tensor_copy(vtb[:, mw, :], vt_ps[:])

                # pass 2: O[i, j] = sum_w V^T[w, i] * band[w, j], output
                # partition dim permuted (i_out = 4p + t) so partition p
                # holds 4 consecutive output rows
                for t in range(4):
                    o_ps = o_psum.tile([P, W], f32, tag="o")
                    for k, lo, hi, start, stop in MM_PLAN_BLOCK:
                        nc.tensor.matmul(
                            o_ps[:, lo:hi],
                            vtb[:, k, t:H:4],
                            band_t[:, k, lo:hi],
                            start=start,
                            stop=stop,
                        )
                    if t % 2 == 0:
                        nc.vector.tensor_copy(o_sb[:, ci, t, :], o_ps[:])
                    else:
                        nc.scalar.copy(o_sb[:, ci, t, :], o_ps[:])

            # one 16KB-per-partition store for the group (ACT HWDGE ring)
            nc.scalar.dma_start(out_ap[g], o_sb[:])


def build_nc():
    import concourse.mybir as mybir
    import concourse.tile as tile
    from concourse import bacc

    # Bacc (not raw Bass): its compile() runs generate_event_semaphores,
    # which splits multi-semaphore waits — walrus codegen allows at most
    # one wait command per DMA instruction.
    nc = bacc.Bacc()
    x = nc.dram_tensor(
        "x", [NG, P, G, NKH, W], mybir.dt.bfloat16, kind="ExternalInput"
    )
    band = nc.dram_tensor("band", [H, W], mybir.dt.bfloat16, kind="ExternalInput")
    out = nc.dram_tensor(
        "out", [NG, P, G, 4, W], mybir.dt.bfloat16, kind="ExternalOutput"
    )
    with tile.TileContext(nc) as tc:
        build_avgpool(tc, x.ap(), band.ap(), out.ap())
    nc.compile()
    return nc


def _ensure_axon_ntff_hook():
    """If tracing is requested (BASS_TRACE) under axon, run_bass_kernel_spmd
    imports antenv.axon_hooks, which some agent images lack. Install the
    real hook if possible, else a stub that degrades tracing gracefully."""
    import sys
    import types

    try:
        import antenv.axon_hooks  # noqa: F401

        return
    except Exception:
        pass
    try:
        import antenv
    except Exception:
        return
    mod = types.ModuleType("antenv.axon_hooks")
    mod._hook = None
    mod.set_axon_ntff_profile_hook = lambda h: setattr(mod, "_hook", h)
    mod.get_axon_ntff_profile_hook = lambda: mod._hook
    sys.modules["antenv.axon_hooks"] = mod
    antenv.axon_hooks = mod
    try:
        from trn_agent_boot.trn_boot import _ntff_profile_via_ctypes

        hook = _ntff_profile_via_ctypes("/opt/axon/libaxon_pjrt.so")
        if hook is not None:
            mod.set_axon_ntff_profile_hook(hook)
    except Exception:
        pass


def prep_inputs(x: np.ndarray):
    """Shard, cast, and pre-block the full input for the 8 cores.

    Device x layout: x_dev[g, p, ci, k, w] = x[4g+ci, 128k+p, w]
    """
    x = np.asarray(x, dtype=np.float32)
    assert x.shape == (8, C, H, W)
    xb = x.astype(ml_dtypes.bfloat16)
    band = make_band()
    in_maps = []
    for b in range(8):
        xd = np.ascontiguousarray(
            xb[b].reshape(NG, G, NKH, P, W).transpose(0, 3, 1, 2, 4)
        )
        in_maps.append({"x": xd, "band": band})
    return in_maps


def gather_output(results) -> np.ndarray:
    """Unshard, un-block, and upcast the per-core bf16 outputs.

    Device out layout: out_dev[g, p, ci, t, w] = out[4g+ci, 4p+t, w]
    """
    full = np.empty((8, C, H, W), dtype=np.float32)
    for b, r in enumerate(results):
        od = np.asarray(r["out"])  # [NG, P, G, 4, W] bf16
        full[b] = (
            od.transpose(0, 2, 1, 3, 4).reshape(C, H, W).astype(np.float32)
        )
    return full


def kernel(x) -> np.ndarray:
    _ensure_axon_ntff_hook()
    from concourse.bass_utils import run_bass_kernel_spmd

    nc = build_nc()
    in_maps = prep_inputs(x)
    res = run_bass_kernel_spmd(nc, in_maps, core_ids=list(range(8)))
    return gather_output(res.results)
